# revision 30
# baseline (speedup 1.0000x reference)
"""Multi-head attention (B=2, S=2048, D=1024, H=16) on 8 trn2 NeuronCores.

Sharding: core c handles batch c//4 and head-group c%4 (4 heads, dh'=256
slice of the projection dims).  Each core computes its heads' Q/K/V
projections, transposed-layout attention (scores as [keys, q] so softmax-exp
is a plain ACT pass and A@V contracts keys on partitions), and a partial
output projection against its Wo column slice.  The host sums the 4 partials
per batch and adds bo.

Schedule (v2): the Scalar-engine exp stream is the steady-state pacer
(~1.1us per key chunk); everything else is sliced to fit underneath it:
- AV matmuls trail their scores by TWO key chunks so the PE never sits on
  the exp->mask->AV dependency chain;
- the mask multiply is one DVE op per key chunk ([128, 2, 512] with the
  mask broadcast across the head pair via a stride-0 free dim, keeping the
  16-bit 2x DVE mode);
- softmax normalization uses reciprocal_approx_fast plus a gpsimd
  partition_broadcast (no DRAM bounce), and the normalize multiplies run
  on the otherwise-idle Pool engine;
- the output projection is split by head pair: pair 0's partial runs
  inside pair 1's key loop (PSUM -> y_sb bf16 copy), pair 1's partial is
  accumulated into y_sb during the NEXT tile's pair-0 loop, so only the
  last pair's 8 matmuls + drain are tail-exposed;
- weights/activations are bf16 throughout (fp8 would be 2x on the PE but
  its ~6% element error lands 1:1 in the output - attention is a
  cancelling sum, errors do not average out - blowing the 2e-2 budget);
- k/v/out-proj chains that share the single spare PSUM bank are emitted as
  complete accumulation chains per slot (never interleaved mid-chain);
- startup: the first k/q projections consume per-chunk weight/input DMAs
  as they land (subtile deps), issues spread across the SP/DVE/ACT
  sequencers so descriptor generation is not serialized on one engine.
"""

import os
import sys

for _p in ("/opt/trn_rl_repo",):
    if _p not in sys.path and os.path.isdir(_p):
        sys.path.insert(0, _p)

import ml_dtypes
import numpy as np

import concourse.bass as bass
import concourse.mybir as mybir
import concourse.tile as tile
from concourse.vector_clock import ScopedClock
from concourse.bass_utils import run_bass_kernel_spmd


def _ensure_axon_hooks_stub():
    """bass_utils imports antenv.axon_hooks when BASS_TRACE=1 under axon;
    this image lacks the module.  Provide a no-hook stub (tracing is then
    skipped gracefully) unless a real one is already installed."""
    try:
        import antenv.axon_hooks  # noqa: F401
    except ImportError:
        import types

        import antenv

        mod = types.ModuleType("antenv.axon_hooks")
        mod._hook = None
        mod.set_axon_ntff_profile_hook = lambda h: setattr(mod, "_hook", h)
        mod.get_axon_ntff_profile_hook = lambda: mod._hook
        sys.modules["antenv.axon_hooks"] = mod
        antenv.axon_hooks = mod


_ensure_axon_hooks_stub()

F32 = mybir.dt.float32
BF16 = mybir.dt.bfloat16
U8 = mybir.dt.uint8
EXP = mybir.ActivationFunctionType.Exp
MUL = mybir.AluOpType.mult
ADD = mybir.AluOpType.add

B, S, D, H, DH = 2, 2048, 1024, 16, 64
NCORES = 8
GH = 4            # heads per core
GD = GH * DH      # 256, dh' slice per core
P = 128
NDC = D // P      # 8 contraction chunks
NQT = 4           # 512-wide query tiles
QT = 512
NKC = S // P      # 16 key chunks
NTT = S // P      # 16 token tiles


# ---------------------------------------------------------------------------
# Walrus-compat shims: this neuronxcc build encodes at most ONE sync wait per
# instruction; Tile's wait assigner emits more.  Hoist overflow waits onto
# injected same-engine NOPs placed immediately before the instruction.
# ---------------------------------------------------------------------------
class _TC(tile.TileContext):
    def _drain_and_barrier(self, tick_clock, wait_clock):
        carrier = self.nc.sync.nop(nofuse=True, hint="tail_waits")
        wait_clock.add_sem_waits(
            carrier.ins, ScopedClock({None: tick_clock.global_clock})
        )
        si = carrier.ins.sync_info
        evs = list(si.on_wait) if si is not None else []
        carrier.ins.sync_info = mybir.SyncInfo(on_wait=evs[:1], on_update=[])
        for k in range(1, len(evs)):
            w = self.nc.sync.nop(nofuse=True, hint=f"tail_wait_{k}")
            w.ins.sync_info = mybir.SyncInfo(on_wait=[evs[k]], on_update=[])
        self.nc.sync.drain()
        self.nc.all_engine_barrier()
        assert self.sems is not None
        popped = self.nc._tile_sem_poison_stack.pop()
        assert popped is self._sem_poison
        self.nc.clear_and_free_semaphores(list(self.sems.allocated().values()))
        self.nc.all_engine_barrier()


def _split_excess_waits(nc: bass.Bass) -> int:
    n_split = 0
    uid = 0
    for f in nc.m.functions:
        for bb in f.blocks:
            new_insts = []
            for inst in bb.instructions:
                si = inst.sync_info
                waits = list(si.on_wait) if si is not None else []
                if len(waits) > 1:
                    for ev in waits[:-1]:
                        nop = mybir.InstNoOp(
                            name=f"I-waitsplit-{uid}", ins=[], outs=[]
                        )
                        uid += 1
                        nop.engine = inst.engine
                        nop.bass_nofuse = True
                        nop.sync_info = mybir.SyncInfo(
                            on_wait=[ev], on_update=[]
                        )
                        new_insts.append(nop)
                        n_split += 1
                    inst.sync_info = mybir.SyncInfo(
                        on_wait=waits[-1:], on_update=list(si.on_update)
                    )
                new_insts.append(inst)
            bb.instructions = new_insts
    return n_split


# ---------------------------------------------------------------------------
# Device kernel (identical on all 8 cores; only the input data differs)
# ---------------------------------------------------------------------------
def _build_nc() -> bass.Bass:
    nc = bass.Bass("TRN2", target_bir_lowering=False)

    qT = nc.dram_tensor("qT", [D, S], BF16, kind="ExternalInput")
    kT = nc.dram_tensor("kT", [D, S], BF16, kind="ExternalInput")
    vT = nc.dram_tensor("vT", [D, S], BF16, kind="ExternalInput")
    maskT = nc.dram_tensor("maskT", [S, S], U8, kind="ExternalInput")
    # weights ship pre-arranged on the host to [P, NDC*GD] / [P, 2*D] so the
    # load is one contiguous line per partition (descriptor-cheap)
    wqT = nc.dram_tensor("wqT", [P, NDC * GD], BF16, kind="ExternalInput")
    wkT = nc.dram_tensor("wkT", [P, NDC * GD], BF16, kind="ExternalInput")
    wvT = nc.dram_tensor("wvT", [P, NDC * GD], BF16, kind="ExternalInput")
    bq = nc.dram_tensor("bq", [GD], F32, kind="ExternalInput")
    bk = nc.dram_tensor("bk", [GD], F32, kind="ExternalInput")
    bv = nc.dram_tensor("bv", [GD], F32, kind="ExternalInput")
    woT = nc.dram_tensor("woT", [P, 2 * D], BF16, kind="ExternalInput")
    selT = nc.dram_tensor("selT", [66, P], BF16, kind="ExternalInput")
    y = nc.dram_tensor("y", [S, D], BF16, kind="ExternalOutput")

    with _TC(nc) as tc:
        with tc.tile_pool(name="persist", bufs=1) as pp:
            # ---- persistent SBUF state ----
            wq_s = pp.tile([P, NDC, GD], BF16)
            wk_s = pp.tile([P, NDC, GD], BF16)
            wv_s = pp.tile([P, NDC, GD], BF16)
            bq_s = pp.tile([P, 2], F32)
            bk_s = pp.tile([P, 2], F32)
            bv_b = pp.tile([P, GD], F32)
            woT_s = pp.tile([P, 2, D], BF16)

            sel_s = pp.tile([66, P], BF16)
            xk_f = pp.tile([P, NDC, S], BF16)   # full-length k input
            xv_f = pp.tile([P, NDC, S], BF16)   # full-length v input
            qpT = pp.tile([P, 2, S], BF16)   # [dh' within pair, pair, tok]
            kpT = pp.tile([P, 2, S], BF16)
            vp_aug = pp.tile([P, NKC, GH, DH + 1], BF16)
            # mask columns for one query tile; two buffers so the next
            # tile's cast-DMA lands while this tile runs
            maskf2 = pp.tile([P, 2, NKC, QT], BF16)

            # k weights + k input feed the first matmuls: issue their DMAs
            # first, split in chunks over three sequencers, so the PE can
            # start as early as possible.
            wkT_r = wkT[:].rearrange("p (c m) -> p c m", c=NDC)
            nc.sync.dma_start(wk_s[:, 0:4, :], wkT_r[:, 0:4, :])
            nc.scalar.dma_start(wk_s[:, 4:8, :], wkT_r[:, 4:8, :])
            nc.scalar.dma_start(bk_s[:], bk[:].rearrange("(c p) -> p c", p=P))

            nc.vector.memset(vp_aug[:, :, :, DH], 1.0)
            nc.scalar.dma_start(sel_s[:], selT[:])

            with (
                tc.tile_pool(name="xa", bufs=3) as xa,
                tc.tile_pool(name="eb", bufs=5) as eb,
                tc.tile_pool(name="nrm", bufs=2) as nrm,
                tc.tile_pool(name="cc", bufs=4) as cc,
                tc.tile_pool(name="yc", bufs=2) as yc,
                tc.tile_pool(name="psA", bufs=1, space="PSUM") as psA,
                tc.tile_pool(name="psS", bufs=2, space="PSUM") as psS,
                tc.tile_pool(name="psAV", bufs=2, space="PSUM") as psAV,
                tc.tile_pool(name="psY", bufs=1, space="PSUM") as psY,
            ):
                # ---------------- projection helpers -----------------------
                # k/v ship into full-length SBUF tiles ([P, NDC, S]; the
                # DRAM lines are 4KB/partition) in per-quarter DMAs so the
                # first projections start after ~1MB has landed.
                x_tiles = {}

                def _xf_dma(dst, src_t, quarter, eng=None, nq=1):
                    s = src_t[:].rearrange("(c p) t -> p c t", p=P)
                    q0 = quarter * QT
                    e1, e2 = eng or (nc.sync, nc.sync)
                    e1.dma_start(
                        dst[:, 0:4, q0 : q0 + nq * QT],
                        s[:, 0:4, q0 : q0 + nq * QT],
                    )
                    e2.dma_start(
                        dst[:, 4:8, q0 : q0 + nq * QT],
                        s[:, 4:8, q0 : q0 + nq * QT],
                    )

                def _x_dma(which, src, qn, split=2, engines=None):
                    """load a [P, NDC, QT] activation tile in chunks"""
                    x_t = xa.tile([P, NDC, QT], BF16, tag="x",
                                  name=f"x{which}{qn}")
                    s = src[:].rearrange("(c p) t -> p c t", p=P)[
                        :, :, qn * QT : (qn + 1) * QT
                    ]
                    step = NDC // split
                    engines = engines or [nc.sync] * split
                    for i in range(split):
                        engines[i % len(engines)].dma_start(
                            x_t[:, i * step : (i + 1) * step, :],
                            s[:, i * step : (i + 1) * step, :],
                        )
                    x_tiles[(which, qn)] = x_t

                proj_ps = {}

                def _proj_chain(which, w_s, b_s, dst, qn, pc, dcs, bias):
                    """proj matmuls for chunks `dcs` (+ bias drain at end).

                    Chains that share psA must never interleave mid-chain:
                    callers emit complete chains, or split one chain across
                    consecutive slots with no other psA user between."""
                    key = (which, qn, pc)
                    if which == "k":
                        x_t = xk_f[:, :, qn * QT : (qn + 1) * QT]
                    else:
                        x_t = x_tiles[(which, qn)][:]
                    if dcs[0] == 0:
                        proj_ps[key] = psA.tile(
                            [P, QT], F32, tag="proj", name=f"ps{which}{qn}{pc}"
                        )
                    ps = proj_ps[key]
                    for dc in dcs:
                        nc.tensor.matmul(
                            ps[:],
                            w_s[:, dc, pc * P : (pc + 1) * P],
                            x_t[:, dc, :],
                            start=(dc == 0),
                            stop=(dc == NDC - 1),
                        )
                    if bias:
                        nc.vector.tensor_scalar_add(
                            dst[:, pc, qn * QT : (qn + 1) * QT],
                            ps[:],
                            b_s[:, pc : pc + 1],
                        )

                def _vproj_tt(tt):
                    """project one 128-token tile of v into vp_aug[:, tt]
                    (complete psA chain)"""
                    ps = psA.tile([P, GD], F32, tag="proj", name=f"psv{tt}")
                    for dc in range(NDC):
                        nc.tensor.matmul(
                            ps[:],
                            xv_f[:, dc, tt * P : (tt + 1) * P],
                            wv_s[:, dc, :],
                            start=(dc == 0),
                            stop=(dc == NDC - 1),
                        )
                    nc.vector.tensor_tensor(
                        vp_aug[:, tt, :, 0:DH],
                        ps[:].rearrange("p (h d) -> p h d", h=GH),
                        bv_b[:].rearrange("p (h d) -> p h d", h=GH),
                        ADD,
                    )

                # ---------------- normalization ----------------------------
                # av_sb: [dh+1, h2, q] f32 per (qt, pair).  The sums rows are
                # DMA-gathered onto partitions {0,32,64,96} of a per-qt s4
                # tile; one DVE reciprocal (bf16 out) covers the whole tile
                # (the DVE reciprocal lowering costs ~6.5ns/free-elem, so it
                # must be amortized per qt); a PE ones-matmul broadcasts each
                # row into a recycled psAV bank; DVE multiplies into concatT.
                av_sbs = {}
                s4s = {}
                r4bs = {}
                rbs = {}
                concatT = {}
                y_sb = {}

                def _drain_av(qt, pair, avs):
                    av_sb = nrm.tile([DH + 1, 2, QT], F32, tag="av_sb",
                                     name=f"avsb{qt}_{pair}")
                    for h2 in range(2):
                        nc.vector.tensor_copy(
                            av_sb[:, h2, :], avs[h2][0 : DH + 1, :]
                        )
                        row = 64 * pair + h2
                        nc.sync.dma_start(
                            s4s[qt][row : row + 1, :],
                            av_sb[DH : DH + 1, h2, :],
                        )
                    av_sbs[(qt, pair)] = av_sb

                def _norm_recip(qt, rows, chunk):
                    """reciprocal of one free-dim chunk of the gathered sums
                    (the DVE reciprocal costs ~6.5ns per free element, so it
                    is sliced across slots to never stall the mask stream)"""
                    if qt not in r4bs:
                        r4bs[qt] = nrm.tile([66, QT], BF16, tag="r4b",
                                            name=f"r4b{qt}")
                    lo, hi = chunk
                    with nc.allow_low_precision(reason="softmax recip via bf16 PE broadcast"):
                        nc.vector.reciprocal(
                            r4bs[qt][rows[0] : rows[1], lo:hi],
                            s4s[qt][rows[0] : rows[1], lo:hi],
                        )

                def _norm_bcast(qt, pair, pool=None):
                    """one K=2 selector matmul broadcasts both heads' recips
                    into a [128, q] PSUM tile (rows 0-63 h2=0, 64-127 h2=1)"""
                    row = 64 * pair
                    pool, tag = pool or (psY, "yp")
                    rb = pool.tile([P, QT], F32, tag=tag,
                                   name=f"rb{qt}_{pair}")
                    nc.tensor.matmul(
                        rb[:],
                        sel_s[row : row + 2, :],
                        r4bs[qt][row : row + 2, :],
                    )
                    rbs[(qt, pair)] = rb

                def _norm_mult(qt, pair, h2):
                    rb = rbs[(qt, pair)]
                    if h2 == 1:
                        del rbs[(qt, pair)]
                    nc.vector.tensor_tensor(
                        concatT[(qt, pair)][64 * h2 : 64 * h2 + 64, :],
                        av_sbs[(qt, pair)][0:DH, h2, :],
                        rb[64 * h2 : 64 * h2 + 64, :],
                        MUL,
                    )

                # ---------------- out-projection ---------------------------
                def _cproj_mm(qt, tt, nh, drain=None):
                    """out-projection for token tile tt, half nh: both pairs
                    accumulate in PSUM, then one drain copy to y_sb"""
                    yp = psY.tile([P, QT], F32, tag="yp",
                                  name=f"yp{qt}_{tt}_{nh}")
                    for pair in range(2):
                        nc.tensor.matmul(
                            yp[:],
                            concatT[(qt, pair)][:, (tt % 4) * P : (tt % 4 + 1) * P],
                            woT_s[:, pair, nh * QT : (nh + 1) * QT],
                            start=(pair == 0),
                            stop=(pair == 1),
                        )
                    if tt not in y_sb:
                        y_sb[tt] = yc.tile([P, 2, QT], BF16, tag="y",
                                           name=f"ysb{tt}", bufs=4)
                    (drain or nc.vector.tensor_copy)(
                        y_sb[tt][:, nh, :], yp[:]
                    )

                def _y_dma(qt, tt):
                    nc.sync.dma_start(
                        y[tt * P : (tt + 1) * P, :],
                        y_sb[tt][:].rearrange("p a b -> p (a b)"),
                    )

                def _mask_dma(qt, chunks=range(4)):
                    """cast-DMA one query tile's mask columns (4 chunks)"""
                    src = maskT[:, qt * QT : (qt + 1) * QT].rearrange(
                        "(c p) t -> p c t", p=P
                    )
                    for i in chunks:
                        nc.gpsimd.dma_start(
                            maskf2[:, qt % 2, 4 * i : 4 * i + 4, :],
                            src[:, 4 * i : 4 * i + 4, :],
                        )

                # ---------------- attention inner loop ---------------------
                # The last two AV matmuls and the PSUM drain of each pair are
                # NOT emitted inside its own loop: they ride the first slots
                # of the NEXT phase (so the next pair's scores keep the
                # PE/ACT streams hot across the boundary).
                pending = {}

                def _attn_pair(qt, pair, extras):
                    """key-chunk loop for one (query tile, head pair);
                    extras[kc] holds callables spliced into slot kc."""
                    qsl = slice(qt * QT, (qt + 1) * QT)
                    mbuf = qt % 2
                    avs = [
                        psAV.tile([DH + 1, QT], F32, tag="av",
                                  name=f"av{qt}_{pair}_{i}")
                        for i in range(2)
                    ]
                    pms = {}

                    def _av(kcd):
                        pm2, j = pms.pop(kcd)
                        for h2 in range(2):
                            nc.tensor.matmul(
                                avs[h2][:],
                                vp_aug[:, kcd, 2 * pair + h2, :],
                                pm2[:, j, h2, :],
                                start=(kcd == 0),
                                stop=(kcd == NKC - 1),
                            )

                    ex2 = None
                    for kc in range(NKC):
                        sc = psS.tile([P, 2, QT], F32, tag="sc")
                        for h2 in range(2):
                            lo = 64 * h2
                            nc.tensor.matmul(
                                sc[:, h2, :],
                                kpT[lo : lo + 64, pair, kc * P : (kc + 1) * P],
                                qpT[lo : lo + 64, pair, qsl],
                            )
                        for fn in extras[kc]:
                            fn()
                        if kc >= 4:
                            _av(kc - 4)
                        # exps land pairwise in one tile; the mask multiply
                        # is one DVE op per TWO key chunks (mask broadcast
                        # across the head dim via a stride-0 free dim)
                        if kc % 2 == 0:
                            ex2 = eb.tile([P, 2, 2, QT], BF16, tag="ex",
                                          bufs=2)
                        nc.scalar.activation(ex2[:, kc % 2, :, :], sc[:], EXP)
                        if kc % 2 == 1:
                            pm2 = eb.tile([P, 2, 2, QT], BF16, tag="pm",
                                          bufs=3)
                            mslice = maskf2[:, mbuf, kc - 1 : kc + 1, :]
                            nc.vector.tensor_tensor(
                                pm2[:],
                                ex2[:],
                                mslice[:, :, None, :].to_broadcast(
                                    (P, 2, 2, QT)
                                ),
                                MUL,
                            )
                            pms[kc - 1] = (pm2, 0)
                            pms[kc] = (pm2, 1)
                    pending[(qt, pair)] = (avs, _av)

                def _finish_pair(qt, pair, step):
                    """steps 0-2: trailing AVs; step 3: last AV + drain"""
                    avs, _av = pending[(qt, pair)]
                    if step < 3:
                        _av(NKC - 4 + step)
                    else:
                        _av(NKC - 1)
                        _drain_av(qt, pair, avs)
                        del pending[(qt, pair)]

                # ---------------- startup ---------------------------------
                # DMA priority: wk + k-quarter-0 (first kproj chain), wq +
                # xq0 (first qproj chain), wv + v-quarter-0 (vproj tile 0),
                # mask chunk 0, remaining k/v quarters, then background.
                _xf_dma(xk_f, kT, 0, eng=(nc.sync, nc.scalar))
                wqT_r = wqT[:].rearrange("p (c m) -> p c m", c=NDC)
                nc.gpsimd.dma_start(wq_s[:, 0:4, :], wqT_r[:, 0:4, :])
                nc.scalar.dma_start(wq_s[:, 4:8, :], wqT_r[:, 4:8, :])
                nc.sync.dma_start(bq_s[:], bq[:].rearrange("(c p) -> p c", p=P))
                _proj_chain("k", wk_s, bk_s, kpT, 0, 0, list(range(NDC)), True)
                _x_dma("q", qT, 0, split=4,
                       engines=[nc.sync, nc.scalar, nc.gpsimd, nc.sync])
                wvT_r = wvT[:].rearrange("p (c m) -> p c m", c=NDC)
                nc.gpsimd.dma_start(wv_s[:, 0:4, :], wvT_r[:, 0:4, :])
                nc.scalar.dma_start(wv_s[:, 4:8, :], wvT_r[:, 4:8, :])
                nc.sync.dma_start(bv_b[:], bv[:][None, :].to_broadcast((P, GD)))
                _proj_chain("q", wq_s, bq_s, qpT, 0, 0, list(range(NDC)), True)
                _xf_dma(xv_f, vT, 0, eng=(nc.scalar, nc.gpsimd))
                _mask_dma(0, chunks=[0])
                _xf_dma(xk_f, kT, 1, eng=(nc.sync, nc.sync))
                _mask_dma(0, chunks=[1, 2, 3])
                _xf_dma(xv_f, vT, 1, eng=(nc.gpsimd, nc.sync))
                _xf_dma(xk_f, kT, 2, eng=(nc.sync, nc.scalar), nq=2)
                _xf_dma(xv_f, vT, 2, eng=(nc.gpsimd, nc.sync), nq=2)
                nc.sync.dma_start(
                    woT_s[:], woT[:].rearrange("p (c n) -> p c n", c=2)
                )

                # ---------------- per-(qt, pair) extra schedules -----------
                # Consumer rule: phase (q, 0) finishes (q-1, 1), normalizes
                # (q-1, 0) [plus the per-qt reciprocal] and runs qproj(q)'s
                # pair-1 chain; phase (q, 1) finishes (q, 0), normalizes
                # (q-1, 1), runs the full out-projection of qt q-1, and
                # qproj(q+1)'s pair-0 chain.  qt 3's pair-0 out-projection
                # half runs inside (3, 1); only pair 1's half is tail-exposed.
                def _cproj_pc(qt, pair, tt, nh, drain, pool=None):
                    pool, tag = pool or (psY, "yp")
                    if tt not in y_sb:
                        y_sb[tt] = yc.tile([P, 2, QT], BF16, tag="y",
                                           name=f"ysb{tt}", bufs=4)
                    yp = pool.tile([P, QT], F32, tag=tag,
                                   name=f"ypp{qt}_{pair}_{tt}_{nh}")
                    nc.tensor.matmul(
                        yp[:],
                        concatT[(qt, pair)][:, (tt % 4) * P : (tt % 4 + 1) * P],
                        woT_s[:, pair, nh * QT : (nh + 1) * QT],
                    )
                    if pair == 0:
                        drain(y_sb[tt][:, nh, :], yp[:])
                    else:
                        nc.vector.tensor_tensor(
                            y_sb[tt][:, nh, :],
                            yp[:],
                            y_sb[tt][:, nh, :],
                            ADD,
                        )

                def _sched(qt, pair):
                    ex = [[] for _ in range(NKC)]

                    def at(kc, fn, *a, **kw):
                        ex[kc].append(lambda: fn(*a, **kw))

                    if qt == 0 and pair == 0:
                        # pair-0 k-projections for tiles 1-3 (tile g ready
                        # before its first use at slot 4g), all the
                        # v-projections (tile tt in slot tt, consumed by AV
                        # at tt+3), and the pair-1 q/k-projections needed
                        # from slot 0 of (0, 1); x tiles prefetched ahead.
                        at(2, _proj_chain, "k", wk_s, bk_s, kpT, 1, 0,
                           [0, 1, 2, 3], False)
                        at(3, _proj_chain, "k", wk_s, bk_s, kpT, 1, 0,
                           [4, 5, 6, 7], True)
                        at(5, _proj_chain, "k", wk_s, bk_s, kpT, 2, 0,
                           [0, 1, 2, 3], False)
                        at(6, _proj_chain, "k", wk_s, bk_s, kpT, 2, 0,
                           [4, 5, 6, 7], True)
                        at(9, _proj_chain, "k", wk_s, bk_s, kpT, 3, 0,
                           [0, 1, 2, 3], False)
                        at(10, _proj_chain, "k", wk_s, bk_s, kpT, 3, 0,
                           [4, 5, 6, 7], True)
                        at(11, _proj_chain, "q", wq_s, bq_s, qpT, 0, 1,
                           [0, 1, 2, 3], False)
                        at(12, _proj_chain, "q", wq_s, bq_s, qpT, 0, 1,
                           [4, 5, 6, 7], True)
                        at(13, _proj_chain, "k", wk_s, bk_s, kpT, 0, 1,
                           [0, 1, 2, 3], False)
                        at(14, _proj_chain, "k", wk_s, bk_s, kpT, 0, 1,
                           [4, 5, 6, 7], True)
                        for tt in range(NTT):
                            at(tt, _vproj_tt, tt)
                        return ex

                    if qt == 0 and pair == 1:
                        # pair-1 k-projections for tiles 1-3, each complete
                        # before its first use at slot 4g
                        at(0, _finish_pair, 0, 0, 0)
                        at(1, _finish_pair, 0, 0, 1)
                        at(2, _finish_pair, 0, 0, 2)
                        at(2, _finish_pair, 0, 0, 3)
                        at(0, _mask_dma, 1)
                        at(0, _proj_chain, "k", wk_s, bk_s, kpT, 1, 1,
                           [0, 1, 2, 3], False)
                        at(1, _proj_chain, "k", wk_s, bk_s, kpT, 1, 1,
                           [4, 5, 6, 7], True)
                        at(4, _proj_chain, "k", wk_s, bk_s, kpT, 2, 1,
                           [0, 1, 2, 3], False)
                        at(5, _proj_chain, "k", wk_s, bk_s, kpT, 2, 1,
                           [4, 5, 6, 7], True)
                        at(8, _proj_chain, "k", wk_s, bk_s, kpT, 3, 1,
                           [0, 1, 2, 3], False)
                        at(9, _proj_chain, "k", wk_s, bk_s, kpT, 3, 1,
                           [4, 5, 6, 7], True)
                        at(4, _x_dma, "q", qT, 1, 2)
                        for i in range(4):
                            at(11 + i, _proj_chain, "q", wq_s, bq_s,
                               qpT, 1, 0, [2 * i, 2 * i + 1], i == 3)
                        return ex

                    if pair == 0:
                        at(0, _finish_pair, qt - 1, 1, 0)
                        at(1, _finish_pair, qt - 1, 1, 1)
                        at(2, _finish_pair, qt - 1, 1, 2)
                        at(2, _finish_pair, qt - 1, 1, 3)
                        # qproj(qt) pair-1 chain after the recips (the DVE
                        # has ~450ns/slot of slack under the exp pace, so
                        # the recip chunks go one-per-two-slots)
                        for i in range(4):
                            at(10 + i, _proj_chain, "q", wq_s, bq_s, qpT,
                               qt, 1, [2 * i, 2 * i + 1], i == 3)
                        if qt >= 1:
                            # one recip set covers both pairs' sums rows
                            # (cost is free-size-bound; pair-1 rows were
                            # gathered by the finish at slot 2)
                            for c in range(4):
                                at(3 + 2 * c, _norm_recip, qt - 1, (0, 66),
                                   (128 * c, 128 * c + 128))
                            at(10, _norm_bcast, qt - 1, 0)
                            at(11, _norm_mult, qt - 1, 0, 0)
                            at(12, _norm_mult, qt - 1, 0, 1)
                    else:
                        at(0, _finish_pair, qt, 0, 0)
                        at(1, _finish_pair, qt, 0, 1)
                        at(2, _finish_pair, qt, 0, 2)
                        at(2, _finish_pair, qt, 0, 3)
                        if qt + 1 < NQT:
                            at(0, _mask_dma, qt + 1)
                            at(4, _x_dma, "q", qT, qt + 1, 2)
                            for i in range(4):
                                at(11 + i, _proj_chain, "q", wq_s, bq_s,
                                   qpT, qt + 1, 0, [2 * i, 2 * i + 1], i == 3)
                        if qt >= 1:
                            at(3, _norm_bcast, qt - 1, 1)
                            at(4, _norm_mult, qt - 1, 1, 0)
                            at(5, _norm_mult, qt - 1, 1, 1)
                            for j in range(8):
                                tt, nh = 4 * (qt - 1) + j // 2, j % 2

                                def cp(pq=qt - 1, tt=tt, nh=nh):
                                    _cproj_mm(pq, tt, nh)
                                    if nh == 1:
                                        _y_dma(pq, tt)

                                at(6 + j, cp)
                        if qt == NQT - 1:
                            # last tile's pair-0 normalize (out-proj half
                            # runs in the tail where ACT is free to drain)
                            for c in range(4):
                                at(3 + 2 * c, _norm_recip, qt, (0, 2),
                                   (128 * c, 128 * c + 128))
                            at(10, _norm_bcast, qt, 0)
                            at(11, _norm_mult, qt, 0, 0)
                            at(12, _norm_mult, qt, 0, 1)
                    return ex

                # ---------------- main loop -------------------------------
                for qt in range(NQT):
                    for pair in range(2):
                        concatT[(qt, pair)] = cc.tile(
                            [P, QT], BF16, tag="cc", name=f"cc{qt}_{pair}"
                        )
                    s4s[qt] = nrm.tile([66, QT], F32, tag="s4",
                                       name=f"s4_{qt}")
                    _attn_pair(qt, 0, _sched(qt, 0))
                    _attn_pair(qt, 1, _sched(qt, 1))

                # tail: last pair's AVs/drain; pair-0 out-proj half with
                # ACT drains (the exp stream is over, so the Scalar engine
                # is free) in parallel with the DVE recip/mult chain; then
                # pair 1's half.  PSUM tiles rotate across three pools.
                lq = NQT - 1
                _finish_pair(lq, 1, 0)
                _finish_pair(lq, 1, 1)
                _finish_pair(lq, 1, 2)
                _finish_pair(lq, 1, 3)
                pools = [(psY, "yp"), (psA, "proj"), (psS, "sc")]
                for j in range(8):
                    tt, nh = 4 * lq + j // 2, j % 2
                    _cproj_pc(lq, 0, tt, nh, nc.scalar.copy,
                              pool=pools[j % 3])
                _norm_recip(lq, (64, 66), (0, 256))
                _norm_recip(lq, (64, 66), (256, QT))
                _norm_bcast(lq, 1, pool=(psAV, "av"))
                _norm_mult(lq, 1, 0)
                _norm_mult(lq, 1, 1)
                for j in range(8):
                    tt, nh = 4 * lq + j // 2, j % 2
                    _cproj_pc(lq, 1, tt, nh, nc.vector.tensor_copy,
                              pool=pools[j % 3])
                    if nh == 1:
                        _y_dma(lq, tt)

    _split_excess_waits(nc)
    return nc


_NC = None
LAST_RESULTS = None  # test harness reads exec_time_ns off this


def kernel(q, k, v, mask, Wq, bq, Wk, bk, Wv, bv, Wo, bo):
    global _NC, LAST_RESULTS
    if _NC is None:
        _NC = _build_nc()

    q = np.asarray(q, np.float32)
    k = np.asarray(k, np.float32)
    v = np.asarray(v, np.float32)
    scale = 1.0 / np.sqrt(np.float32(DH))

    bf = ml_dtypes.bfloat16
    qTb = [np.ascontiguousarray(q[b].T.astype(bf)) for b in range(B)]
    kTb = [np.ascontiguousarray(k[b].T.astype(bf)) for b in range(B)]
    vTb = [np.ascontiguousarray(v[b].T.astype(bf)) for b in range(B)]
    maskT_u8 = np.ascontiguousarray(
        np.asarray(mask)[0, 0].T.astype(np.uint8)
    )

    Wq = np.asarray(Wq, np.float32)
    Wk = np.asarray(Wk, np.float32)
    Wv = np.asarray(Wv, np.float32)
    Wo = np.asarray(Wo, np.float32)

    sel = np.zeros((66, P), dtype=ml_dtypes.bfloat16)
    sel[0, 0:64] = 1
    sel[1, 64:128] = 1
    sel[64, 0:64] = 1
    sel[65, 64:128] = 1

    def _warr(wT):  # [D, GD] -> [P, NDC*GD] per-partition-contiguous, bf16
        return np.ascontiguousarray(
            wT.reshape(NDC, P, GD)
            .transpose(1, 0, 2)
            .reshape(P, NDC * GD)
            .astype(ml_dtypes.bfloat16)
        )

    in_maps = []
    for c in range(NCORES):
        b, g = divmod(c, NCORES // B)
        rows = slice(GD * g, GD * (g + 1))
        in_maps.append(
            {
                "qT": qTb[b],
                "kT": kTb[b],
                "vT": vTb[b],
                "maskT": maskT_u8,
                "selT": sel,
                "wqT": _warr((Wq[rows] * scale).T),
                "wkT": _warr(Wk[rows].T),
                "wvT": _warr(Wv[rows].T),
                "bq": np.ascontiguousarray(np.asarray(bq, np.float32)[rows] * scale),
                "bk": np.ascontiguousarray(np.asarray(bk, np.float32)[rows]),
                "bv": np.ascontiguousarray(np.asarray(bv, np.float32)[rows]),
                "woT": np.ascontiguousarray(
                    Wo[:, rows].T.reshape(2, P, D)
                    .transpose(1, 0, 2)
                    .reshape(P, 2 * D)
                    .astype(ml_dtypes.bfloat16)
                ),
            }
        )

    res = run_bass_kernel_spmd(_NC, in_maps, core_ids=list(range(NCORES)))
    LAST_RESULTS = res

    ng = NCORES // B
    out = np.empty((B, S, D), np.float32)
    for b in range(B):
        acc = res.results[b * ng]["y"].astype(np.float32)
        for g in range(1, ng):
            acc += res.results[b * ng + g]["y"].astype(np.float32)
        out[b] = acc + np.asarray(bo, np.float32)
    return out
